# revision 19
# baseline (speedup 1.0000x reference)
"""Fused single-head attention (QKV proj + softmax(QK^T)V) on 8 trn2 cores.

Problem (hardcoded): x [4, 4096, 768] f32, W_qkv [768, 2304] f32, b_qkv
[2304] f32:
  qkv = x @ W_qkv + b_qkv ; q,k,v = split(qkv, 3)
  out = softmax(q k^T / sqrt(768)) v          -> [4, 4096, 768] f32

Sharding: batch (4) x key-halves (2) -> 8 cores. Core (b, r) owns rows
[r*2048, (r+1)*2048) of batch b as its keys AND as its share of the
query-side projection; everything stays in global query order. Each
core computes PARTIAL attention sums over its 2048 keys; the host
combines pairs: (o0 + o1) / (d0 + d1). No max-subtraction (scores are
O(1)).

Math (vs a straight fp16 flash kernel, all verified against a numpy
bit-accurate simulation of the pipeline):
  - k-projection is eliminated: scores = x M x^T * SCALE with
    M = W_q W_k^T precomputed on host (fp16, pre-scaled by 32).
  - q-bias cancels in softmax; k-bias folds into a per-key additive
    score bias w_j = (x @ (W_k b_q) + b_q.b_k) * SCALE computed on the
    host and applied inside the exp activation (per-partition bias).
  - v-bias is added by the host after the combine.
  - t = x @ (32 M) computed on-device in fp16 (f32 accum) for the OWN
    query half only, stored fp8; the pair exchanges halves via two
    pairwise DRAM AllGathers (split in two chunks so the first ships
    while the second half is still being computed; both overlap the
    v-projection).
  - scores psum = (32 t) @ x8^T via 3 fp8-e4m3 DoubleRow matmuls
    (256-deep contraction each, 2x PE rate); exp scale folds the 1/32
    and 1/sqrt(H).
  - PV and v-projection stay fp16: fp8 there fails the 2e-2 gate
    (v/p quantization error passes straight to the output).
Attention inner loop: per i-block, pass A does scores + exp + PV for
h-tiles 0-2 with PV running TWO score-groups behind (the ~800ns exp
latency hides under ~1.9us of PE work); pass B replays the stored p16
for h-tiles 3-5 while pass-A accumulators drain.

Measured-on-hw notes: DoubleRow sustains full rate ONLY when both
operands are sliced from single big SBUF tiles (separate small tiles
rotating per-matmul serialize ~2.3x); f32->fp8 evac on ScalarE/DVE is
exact round-to-nearest (required: truncation would double the error).

Layouts (all per-partition-major, single big tiles):
  xx8 [128, 6, 2048] fp8   x^T for this core's keys, c-tile-structured
  tt8 [128, 6, 4096] fp8   (32 t)^T, assembled from the AllGathers
  vv  [128, 16, 768] fp16  v tiles (j-tile-structured)
  p16 [128, 16, 512] fp16  exp(scores) for the current i-block
PSUM phase 2: scores/og-B 3 banks (shared tag) + og-A 3 banks.
"""

import math
from contextlib import ExitStack
from functools import lru_cache

import ml_dtypes
import numpy as np

import concourse.bacc as bacc
import concourse.bass as bass
import concourse.tile as tile
from concourse import mybir
from concourse.bass_utils import run_bass_kernel_spmd

B, N, C = 4, 4096, 768
H = 768
NCORES = 8
NK = N // 2      # keys per core
DT = mybir.dt.float16
DT8 = mybir.dt.float8e4
F32 = mybir.dt.float32
SCALE = 1.0 / math.sqrt(H)
TSCALE = 32.0    # host pre-scales M by this; exp folds 1/TSCALE back out
DRMODE = mybir.MatmulPerfMode.DoubleRow

CT = C // 128    # 6 contraction tiles
HT = H // 128    # 6 head tiles
JT = NK // 128   # 16 key tiles per core
RB = 8           # r-blocks of 512 over the 4096 rows
RBS = N // RB    # 512
KRB = RB // 2    # r-blocks containing this core's keys (first 4)
IB = 8
IBS = N // IB    # 512
NP8 = ml_dtypes.float8_e4m3


def build_program():
    nc = bacc.Bacc(
        "TRN2",
        target_bir_lowering=False,
        debug=False,
        enable_asserts=False,
        num_devices=NCORES,
    )
    # All tensors are in GLOBAL query order. Core (b, r) owns rows
    # [r*NK, (r+1)*NK) of batch b as BOTH its keys and its t-projection
    # share; the pair AllGathers the fp8 t-halves so neither core computes
    # the full 4096-query projection.
    x16_d = nc.dram_tensor("x16", [128, CT, NK], DT, kind="ExternalInput").ap()
    xx8_d = nc.dram_tensor("xx8", [128, CT, NK], DT8, kind="ExternalInput").ap()
    m16_d = nc.dram_tensor("m16", [128, CT, H], DT, kind="ExternalInput").ap()
    wv16_d = nc.dram_tensor("wv16", [128, CT, H], DT, kind="ExternalInput").ap()
    wj_d = nc.dram_tensor("wj", [128, JT], F32, kind="ExternalInput").ap()
    outT_d = nc.dram_tensor("outT", [H, N], DT, kind="ExternalOutput").ap()
    # per-partition partial softmax denominators; host sums over axis 1
    den_d = nc.dram_tensor("den", [IB, 128, IBS], DT, kind="ExternalOutput").ap()

    with tile.TileContext(nc) as tc:
        with ExitStack() as ctx:
            persist = ctx.enter_context(tc.tile_pool(name="persist", bufs=1))

            xx8 = persist.tile([128, CT, NK], DT8, tag="xx8")
            tt8 = persist.tile([128, CT, N], DT8, tag="tt8")
            t8own = persist.tile([128, CT, NK], DT8, tag="t8own")
            vv = persist.tile([128, JT, H], DT, tag="vv")
            p16 = persist.tile([128, JT, IBS], DT, tag="p16")
            wj = persist.tile([128, JT], F32, tag="wj")
            m16 = persist.tile([128, CT, H], DT, tag="m16")
            wv16 = persist.tile([128, CT, H], DT, tag="wv16")

            # ---- Phase 1: t-proj (own half, fp16) + AllGather + v-proj ----
            # bufs=4: all four x blocks stay live across the split
            # t-proj / v-proj loops
            with tc.tile_pool(name="xpool", bufs=4) as xpool, \
                 tc.tile_pool(name="dram", bufs=1, space="DRAM") as dram, \
                 tc.tile_pool(name="pj", bufs=4, space="PSUM") as pj, \
                 tc.tile_pool(name="pv", bufs=2, space="PSUM") as pv:

                # asymmetric AllGather chunks: a small first chunk (r-block 0
                # only) ships as early as possible to cover phase-2's first
                # i-block; the rest follows in one bigger transfer.
                CHW = [RBS, NK - RBS]          # chunk widths: 512, 1536
                CH0 = [0, RBS]                 # chunk column offsets
                t8halves = [dram.tile([128, CT, CHW[i]], DT8, tag=f"t8h{i}",
                                      name=f"t8h{i}") for i in range(2)]
                t8gaths = [dram.tile([2, 128, CT, CHW[i]], DT8, tag=f"t8g{i}",
                                     name=f"t8g{i}") for i in range(2)]

                def load_xt(rb):
                    r0 = rb * RBS
                    t = xpool.tile([128, CT, RBS], DT, tag="xt",
                                   name=f"xt{rb}")
                    for ct in range(CT):
                        nc.sync.dma_start(out=t[:, ct, :],
                                          in_=x16_d[:, ct, r0:r0 + RBS])
                    return t

                # DMA issue order = need order: interleave M c-tiles with the
                # first r-block's x so the ct=0 matmul's inputs arrive first.
                xts = [None] * KRB
                xt0 = xpool.tile([128, CT, RBS], DT, tag="xt", name="xt0")
                for ct in range(CT):
                    nc.sync.dma_start(out=m16[:, ct, :], in_=m16_d[:, ct, :])
                    nc.sync.dma_start(out=xt0[:, ct, :],
                                      in_=x16_d[:, ct, 0:RBS])
                xts[0] = xt0
                for ct in range(CT):
                    nc.sync.dma_start(out=wv16[:, ct, :], in_=wv16_d[:, ct, :])
                nc.sync.dma_start(out=wj, in_=wj_d)
                nc.sync.dma_start(out=xx8, in_=xx8_d)

                # PE warm-up: junk matmuls (no DMA deps) so the HAM clock-gate
                # reaches full rate while the first M/x DMAs are in flight.
                warm_l = xpool.tile([128, 128], DT, tag="warml", name="warml")
                warm_r = xpool.tile([128, 512], DT, tag="warmr", name="warmr")
                nc.vector.memset(warm_l, 0.0)
                nc.vector.memset(warm_r, 0.0)
                for i in range(16):
                    wp = pj.tile([128, RBS], F32, tag="pj", name=f"warm{i}")
                    nc.tensor.matmul(wp, warm_l, warm_r, start=True, stop=True)

                # t-proj for the own query half only (local rbs 0..3)
                for rb in range(KRB):
                    r0 = rb * RBS
                    if rb + 1 < KRB:
                        xts[rb + 1] = load_xt(rb + 1)
                    xt = xts[rb]
                    for ht in range(HT):
                        ps = pj.tile([128, RBS], F32, tag="pj")
                        for ct in range(CT):
                            nc.tensor.matmul(
                                ps,
                                m16[:, ct, ht * 128:(ht + 1) * 128],
                                xt[:, ct, :],
                                start=(ct == 0), stop=(ct == CT - 1),
                            )
                        if ht % 2 == 0:
                            nc.scalar.activation(
                                out=t8own[:, ht, r0:r0 + RBS], in_=ps,
                                func=mybir.ActivationFunctionType.Copy)
                        else:
                            nc.vector.tensor_copy(
                                out=t8own[:, ht, r0:r0 + RBS], in_=ps)
                    ch = 0 if rb == 0 else 1
                    hc0 = r0 - CH0[ch]
                    nc.sync.dma_start(
                        out=t8halves[ch][:, :, hc0:hc0 + RBS],
                        in_=t8own[:, :, r0:r0 + RBS])
                    if rb == 0 or rb == KRB - 1:
                        c0, cw, gath = CH0[ch], CHW[ch], t8gaths[ch]
                        nc.gpsimd.collective_compute(
                            "AllGather",
                            mybir.AluOpType.bypass,
                            replica_groups=[[2 * p, 2 * p + 1]
                                            for p in range(NCORES // 2)],
                            ins=[t8halves[ch].opt()],
                            outs=[gath.opt()],
                        )
                        # assemble full (global-order) t^T as chunks land
                        nc.sync.dma_start(out=tt8[:, :, c0:c0 + cw],
                                          in_=gath[0])
                        nc.sync.dma_start(out=tt8[:, :, NK + c0:NK + c0 + cw],
                                          in_=gath[1])

                # v-proj for the own keys (same local rows)
                for rb in range(KRB):
                    xt = xts[rb]
                    for j in range(RBS // 128):
                        jt = rb * (RBS // 128) + j
                        ps = pv.tile([128, H], F32, tag="pv")
                        for ct in range(CT):
                            xs = xt[:, ct, j * 128:(j + 1) * 128]
                            nc.tensor.matmul(
                                ps[:, 0:512], xs, wv16[:, ct, 0:512],
                                start=(ct == 0), stop=(ct == CT - 1))
                            nc.tensor.matmul(
                                ps[:, 512:H], xs, wv16[:, ct, 512:H],
                                start=(ct == 0), stop=(ct == CT - 1))
                        nc.vector.tensor_copy(out=vv[:, jt, :], in_=ps)

            # ---- Phase 2: attention (partial sums over this core's keys) ----
            # Per i-block, two passes over the 16 j-tiles:
            #   pass A: scores (3 DR mms) + exp + PV for h-tiles 0-2,
            #           with PV-A running TWO score-groups behind so the
            #           823ns exp latency hides under ~1.9us of PE work.
            #   pass B: PV for h-tiles 3-5 from the stored p16 (no deps).
            # PSUM: scores 3 banks (tag "s") + og-A 3 (tag "oa") = 6; pass B's
            # og-B reuses the "s" slots (same size) once scores are drained.
            with tc.tile_pool(name="opool", bufs=8) as opool, \
                 tc.tile_pool(name="spool", bufs=2) as spool, \
                 tc.tile_pool(name="psum2", bufs=1, space="PSUM") as psum2:

                HA = HT // 2     # h-tiles in pass A
                pending = []     # deferred PE work

                def emit_pva(oga, jt):
                    def go():
                        for ht in range(HA):
                            nc.tensor.matmul(
                                oga[ht],
                                vv[:, jt, ht * 128:(ht + 1) * 128],
                                p16[:, jt, :],
                                start=(jt == 0), stop=(jt == JT - 1),
                            )
                    pending.append(go)

                for ib in range(IB):
                    i0 = ib * IBS
                    oga = [psum2.tile([128, IBS], F32, tag="oa", bufs=3,
                                      name=f"oa{ib}_{g}") for g in range(HA)]
                    Sf = spool.tile([128, IBS], F32, tag="Sf", name=f"Sf{ib}")
                    # ---- pass A ----
                    for jt in range(JT):
                        sps = psum2.tile([128, IBS], F32, tag="s", bufs=3)
                        for t in range(CT // 2):
                            nc.tensor.matmul(
                                sps,
                                xx8[:, 2 * t:2 * t + 2, jt * 128:(jt + 1) * 128],
                                tt8[:, 2 * t:2 * t + 2, i0:i0 + IBS],
                                start=(t == 0), stop=(t == CT // 2 - 1),
                                perf_mode=DRMODE,
                            )
                        while len(pending) > 1:
                            pending.pop(0)()
                        nc.scalar.activation(
                            out=p16[:, jt, :], in_=sps,
                            func=mybir.ActivationFunctionType.Exp,
                            scale=SCALE / TSCALE,
                            bias=wj[:, jt:jt + 1],
                        )
                        if jt == 0:
                            nc.vector.tensor_copy(out=Sf, in_=p16[:, jt, :])
                        else:
                            nc.vector.tensor_add(Sf, Sf, p16[:, jt, :])
                        emit_pva(oga, jt)
                    while pending:
                        pending.pop(0)()
                    S16 = spool.tile([128, IBS], DT, tag="S16", name=f"S16{ib}")
                    nc.vector.tensor_copy(out=S16, in_=Sf)
                    nc.sync.dma_start(out=den_d[ib], in_=S16)
                    # og-A evac on DVE (runs while PE does pass B)
                    for ht in range(HA):
                        ot = opool.tile([128, IBS], DT, tag="ot",
                                        name=f"ota{i0}_{ht}")
                        nc.vector.tensor_copy(out=ot, in_=oga[ht])
                        nc.sync.dma_start(
                            out=outT_d[ht * 128:(ht + 1) * 128, i0:i0 + IBS],
                            in_=ot)
                    # ---- pass B (g outer: each og-B bank drains while the
                    # next accumulates, so evacs hide under PE work) ----
                    ogb = [psum2.tile([128, IBS], F32, tag="s", bufs=3,
                                      name=f"ob{ib}_{g}") for g in range(HA)]
                    for g in range(HA):
                        ht = HA + g
                        for jt in range(JT):
                            nc.tensor.matmul(
                                ogb[g],
                                vv[:, jt, ht * 128:(ht + 1) * 128],
                                p16[:, jt, :],
                                start=(jt == 0), stop=(jt == JT - 1),
                            )
                        ot = opool.tile([128, IBS], DT, tag="ot",
                                        name=f"otb{i0}_{g}")
                        nc.vector.tensor_copy(out=ot, in_=ogb[g])
                        nc.sync.dma_start(
                            out=outT_d[ht * 128:(ht + 1) * 128, i0:i0 + IBS],
                            in_=ot)
    nc.compile()
    return nc


@lru_cache(maxsize=1)
def _cached_program():
    return build_program()


def _ctile(a):
    """[C, X] -> [128, CT, X] (c-tile-structured, partition-major)."""
    return np.ascontiguousarray(
        a.reshape(CT, 128, a.shape[1]).transpose(1, 0, 2))


def _prep_in_maps(x, W_qkv, b_qkv):
    x = np.asarray(x, dtype=np.float32)
    W_qkv = np.asarray(W_qkv, dtype=np.float32)
    b_qkv = np.asarray(b_qkv, dtype=np.float32)
    Wq, Wk, Wv = W_qkv[:, :H], W_qkv[:, H:2 * H], W_qkv[:, 2 * H:]
    bq, bk = b_qkv[:H], b_qkv[H:2 * H]

    M32 = _ctile((TSCALE * (Wq @ Wk.T)).astype(np.float16))  # [128, CT, H]
    wv16 = _ctile(Wv.astype(np.float16))
    u = Wk @ bq                                              # [C]
    c0 = float(bq @ bk)

    in_maps = []
    for core in range(NCORES):
        b, kh = core // 2, core % 2
        # global query order everywhere; this core owns rows [kh*NK,(kh+1)*NK)
        xo = x[b][kh * NK:(kh + 1) * NK]                     # [NK, C]
        x16 = _ctile(np.ascontiguousarray(xo.T).astype(np.float16))
        xx8 = _ctile(np.ascontiguousarray(xo.T).astype(NP8))
        wjv = ((xo @ u + c0) * SCALE).astype(np.float32)
        wj = np.ascontiguousarray(wjv.reshape(JT, 128).T)    # [128, JT]
        in_maps.append({"x16": x16, "xx8": xx8, "m16": M32,
                       "wv16": wv16, "wj": wj})
    return in_maps


def _combine(results, b_qkv):
    bv = np.asarray(b_qkv, dtype=np.float32)[2 * H:]
    out = np.empty((B, N, C), dtype=np.float32)
    for b in range(B):
        o0 = results[2 * b]["outT"].astype(np.float32)       # [H, N]
        d0 = results[2 * b]["den"].astype(np.float32).sum(axis=1).reshape(N)
        o1 = results[2 * b + 1]["outT"].astype(np.float32)
        d1 = results[2 * b + 1]["den"].astype(np.float32).sum(axis=1).reshape(N)
        out[b] = ((o0 + o1) / (d0 + d1)).T + bv[None, :]
    return out


def kernel(x, W_qkv, b_qkv):
    nc = _cached_program()
    in_maps = _prep_in_maps(x, W_qkv, b_qkv)
    res = run_bass_kernel_spmd(nc, in_maps, core_ids=list(range(NCORES)))
    return _combine(res.results, b_qkv)
